# revision 3
# baseline (speedup 1.0000x reference)
"""Causal single-head attention (B=4, S=4096, D=1024, fp32) on 8 TRN2 NeuronCores.

Sharding: data-parallel over batch (4) x 2-way causal-balanced query split.
Core c handles batch c//2; role r = c%2 takes global 512-row query blocks
[1,3,5,7] (r=1) or [0,2,4,6] (r=0), assigned to 4 "slots" with uniform
per-slot key-chunk capacities [8,16,24,32] so all 8 cores run one SPMD
program; causality and per-core block offsets are enforced purely by data
(mask thresholds DMA'd per core).

Per-core pipeline (all matmuls on TensorE):
  1) qT = (x@Wq).T, kT = (x@Wk).T  in fp32r (full-rate, ~1e-4 rel err),
     stored bf16 in SBUF; v = x@Wv spilled to DRAM bf16.
  2) per slot: scoresT[key,q] = kT.T-chunks @ qT (bf16), exp via ScalarE
     (scale 1/32) into a bf16 strip, causal mask = (iota >= thr) built on
     VectorE, denominators via ones-matmul (column sums on TensorE),
     out.T[e,q] = sum_s v[s,e] * expT[s,q] accumulated in PSUM, normalized
     by reciprocal(sums) and DMA'd out.
No collectives; host transposes x / assembles the output.
"""
import sys
import numpy as np

sys.path.insert(0, "/opt/trn_rl_repo")

B, S, D = 4, 4096, 1024
P = 128
QB = 512
DC = D // P            # 8 contraction chunks of 128
NSLOT = 4
MAXKC = S // P // 1    # 32
CAPS = [8, 16, 24, 32]
SKIPS = [0, 8, 16, 24]
QBLOCKS = [[0, 2, 4, 6], [1, 3, 5, 7]]   # role -> global 512-block per slot
NCORES = 8
QLOC = NSLOT * QB      # 2048 query rows per core
SCALE = 1.0 / np.sqrt(np.float32(D))     # softmax 1/sqrt(d_out)

_built = None


def _build():
    import concourse.mybir as mybir
    import concourse.tile as tile
    from concourse import bacc

    f32 = mybir.dt.float32
    bf16 = mybir.dt.bfloat16
    f32r = mybir.dt.float32r

    nc = bacc.Bacc("TRN2", target_bir_lowering=False, debug=False,
                   num_devices=NCORES)
    xT = nc.dram_tensor("xT", [D, S], f32r, kind="ExternalInput")
    xTq = nc.dram_tensor("xTq", [D, QLOC], f32r, kind="ExternalInput")
    Wqk = nc.dram_tensor("Wqk", [D, 2 * D], f32r, kind="ExternalInput")
    Wv = nc.dram_tensor("Wv", [D, D], f32r, kind="ExternalInput")
    thr = nc.dram_tensor("thr", [P, NSLOT * MAXKC], f32, kind="ExternalInput")
    iota = nc.dram_tensor("iota", [P, QB], f32, kind="ExternalInput")
    outT = nc.dram_tensor("outT", [D, QLOC], f32, kind="ExternalOutput")

    xT_r = xT.ap().rearrange("(c p) s -> p c s", p=P)
    xTq_r = xTq.ap().rearrange("(c p) s -> p c s", p=P)
    Wqk_r = Wqk.ap().rearrange("(c p) e -> p c e", p=P)
    Wv_r = Wv.ap().rearrange("(c p) e -> p c e", p=P)

    with tile.TileContext(nc) as tc, \
         tc.tile_pool(name="res", bufs=1) as res, \
         tc.tile_pool(name="const", bufs=1) as constp, \
         tc.tile_pool(name="p1small", bufs=3) as p1small, \
         tc.tile_pool(name="dram", bufs=1, space="DRAM") as dramp, \
         tc.tile_pool(name="psA", bufs=4, space="PSUM") as psA, \
         tc.tile_pool(name="psS", bufs=2, space="PSUM") as psS, \
         tc.tile_pool(name="psR", bufs=1, space="PSUM") as psR:

        kT = res.tile([P, DC, S], bf16, tag="kT")
        qT = res.tile([P, DC, QLOC], bf16, tag="qT")
        vsp = dramp.tile([S, D], bf16, tag="vsp")

        iota_sb = constp.tile([P, QB], f32, tag="iota")
        thr_sb = constp.tile([P, NSLOT * MAXKC], f32, tag="thr")
        ones_sb = constp.tile([P, P], bf16, tag="ones")
        nc.sync.dma_start(out=iota_sb[:], in_=iota.ap())
        nc.sync.dma_start(out=thr_sb[:], in_=thr.ap())
        nc.vector.memset(ones_sb[:], 1.0)

        # ---------------- phase 1: projections (fp32r) ----------------
        with tc.tile_pool(name="w", bufs=1) as wpool, \
             tc.tile_pool(name="xs", bufs=2) as xs:
            wqk_sb = wpool.tile([P, DC, 2 * D], f32r, tag="w")
            nc.sync.dma_start(out=wqk_sb[:], in_=Wqk_r)

            def proj_T(dst, src_r, nblocks, w_off):
                # dst[:, ec, blk] = sum_dc W[dc, w_off+ec].T @ xT[dc, blk]
                for blk in range(nblocks):
                    xstrip = xs.tile([P, DC, QB], f32r, tag="xs")
                    nc.sync.dma_start(
                        out=xstrip[:],
                        in_=src_r[:, :, blk * QB:(blk + 1) * QB])
                    for ec in range(DC):
                        acc = psA.tile([P, QB], f32, tag="acc")
                        for dc in range(DC):
                            nc.tensor.matmul(
                                acc[:],
                                lhsT=wqk_sb[:, dc,
                                            w_off + ec * P:w_off + (ec + 1) * P],
                                rhs=xstrip[:, dc],
                                start=(dc == 0), stop=(dc == DC - 1))
                        d = dst[:, ec, blk * QB:(blk + 1) * QB]
                        if ec % 2 == 0:
                            nc.vector.tensor_copy(d, acc[:])
                        else:
                            nc.scalar.copy(d, acc[:])

            proj_T(qT, xTq_r, QLOC // QB, 0)       # Wq columns [0, D)
            proj_T(kT, xT_r, S // QB, D)           # Wk columns [D, 2D)

            # v = x @ Wv  -> vsp (s-major bf16, spilled to DRAM)
            wv_sb = wpool.tile([P, DC, D], f32r, tag="w")
            nc.sync.dma_start(out=wv_sb[:], in_=Wv_r)
            for blk in range(S // QB):
                xstrip = xs.tile([P, DC, QB], f32r, tag="xs")
                nc.sync.dma_start(
                    out=xstrip[:], in_=xT_r[:, :, blk * QB:(blk + 1) * QB])
                for ss in range(QB // P):          # 4 s-subchunks
                    for eb in range(D // QB):      # 2 e-halves
                        acc = psA.tile([P, QB], f32, tag="acc")
                        for dc in range(DC):
                            nc.tensor.matmul(
                                acc[:],
                                lhsT=xstrip[:, dc, ss * P:(ss + 1) * P],
                                rhs=wv_sb[:, dc, eb * QB:(eb + 1) * QB],
                                start=(dc == 0), stop=(dc == DC - 1))
                        vtmp = p1small.tile([P, QB], bf16, tag="vtmp")
                        if (ss + eb) % 2 == 0:
                            nc.vector.tensor_copy(vtmp[:], acc[:])
                        else:
                            nc.scalar.copy(vtmp[:], acc[:])
                        r0 = blk * QB + ss * P
                        nc.sync.dma_start(
                            out=vsp[r0:r0 + P, eb * QB:(eb + 1) * QB],
                            in_=vtmp[:])

        # ---------------- phase 2: attention ----------------
        with tc.tile_pool(name="expp", bufs=2) as expp, \
             tc.tile_pool(name="vs", bufs=4) as vs, \
             tc.tile_pool(name="p2small", bufs=2) as p2s:
            for j in range(NSLOT):
                cap, skip = CAPS[j], SKIPS[j]
                expT = expp.tile([P, MAXKC, QB], bf16, tag="expT")
                # scoresT -> exp -> mask
                for kc in range(cap):
                    sc = psS.tile([P, QB], f32, tag="sc")
                    for ec in range(DC):
                        nc.tensor.matmul(
                            sc[:],
                            lhsT=kT[:, ec, kc * P:(kc + 1) * P],
                            rhs=qT[:, ec, j * QB:(j + 1) * QB],
                            start=(ec == 0), stop=(ec == DC - 1))
                    nc.scalar.activation(
                        expT[:, kc], sc[:],
                        func=mybir.ActivationFunctionType.Exp,
                        scale=float(SCALE))
                    if kc >= skip:
                        m = p2s.tile([P, QB], bf16, tag="mask")
                        nc.vector.tensor_scalar(
                            m[:], iota_sb[:],
                            thr_sb[:, j * MAXKC + kc:j * MAXKC + kc + 1],
                            None, mybir.AluOpType.is_ge)
                        nc.vector.tensor_mul(expT[:, kc], expT[:, kc], m[:])
                # denominators (column sums via ones-matmul)
                ds = psR.tile([P, QB], f32, tag="sum")
                for kc in range(cap):
                    nc.tensor.matmul(ds[:], lhsT=ones_sb[:], rhs=expT[:, kc],
                                     start=(kc == 0), stop=(kc == cap - 1))
                sums_sb = p2s.tile([P, QB], f32, tag="sums")
                nc.vector.tensor_copy(sums_sb[:], ds[:])
                recip = p2s.tile([P, QB], f32, tag="recip")
                nc.vector.reciprocal(recip[:], sums_sb[:])
                # out.T accumulation, e in two halves of 4 chunks
                for half in range(2):
                    accs = [psA.tile([P, QB], f32, tag="acc",
                                     name=f"acc_{j}_{half}_{i}")
                            for i in range(4)]
                    for kc in range(cap):
                        vh = vs.tile([P, QB], bf16, tag="vh")
                        nc.sync.dma_start(
                            out=vh[:],
                            in_=vsp[kc * P:(kc + 1) * P,
                                    half * QB:(half + 1) * QB])
                        for e4 in range(4):
                            nc.tensor.matmul(
                                accs[e4][:],
                                lhsT=vh[:, e4 * P:(e4 + 1) * P],
                                rhs=expT[:, kc],
                                start=(kc == 0), stop=(kc == cap - 1))
                    for e4 in range(4):
                        ot = p2s.tile([P, QB], f32, tag="ot")
                        nc.vector.tensor_mul(ot[:], accs[e4][:], recip[:])
                        r0 = (half * 4 + e4) * P
                        nc.sync.dma_start(
                            out=outT.ap()[r0:r0 + P, j * QB:(j + 1) * QB],
                            in_=ot[:])

    nc.finalize()
    return nc


def _get_nc():
    global _built
    if _built is None:
        _built = _build()
    return _built


def _host_inputs(x, Wq, Wk, Wv):
    iota = np.broadcast_to(
        np.arange(QB, dtype=np.float32), (P, QB)).copy()
    Wqk = np.ascontiguousarray(
        np.concatenate([Wq, Wk], axis=1).astype(np.float32))
    Wv = np.ascontiguousarray(Wv.astype(np.float32))
    p = np.arange(P, dtype=np.float32)[:, None]
    thrs = []
    for role in range(2):
        t = np.zeros((P, NSLOT * MAXKC), np.float32)
        for j in range(NSLOT):
            q0 = QBLOCKS[role][j] * QB
            for kc in range(MAXKC):
                t[:, j * MAXKC + kc] = np.clip(
                    kc * P + p[:, 0] - q0, 0, QB)
        thrs.append(t)
    xTs = [np.ascontiguousarray(np.asarray(x[b]).T.astype(np.float32))
           for b in range(B)]
    in_maps = []
    for c in range(NCORES):
        b, role = divmod(c, 2)
        cols = np.concatenate(
            [np.arange(QBLOCKS[role][j] * QB, QBLOCKS[role][j] * QB + QB)
             for j in range(NSLOT)])
        xTq = np.ascontiguousarray(xTs[b][:, cols])
        in_maps.append({"xT": xTs[b], "xTq": xTq, "Wqk": Wqk, "Wv": Wv,
                        "thr": thrs[role], "iota": iota})
    return in_maps


def _assemble(results):
    out = np.empty((B, S, D), np.float32)
    for c in range(NCORES):
        b, role = divmod(c, 2)
        oT = results[c]["outT"]
        for j in range(NSLOT):
            q0 = QBLOCKS[role][j] * QB
            out[b, q0:q0 + QB, :] = oT[:, j * QB:(j + 1) * QB].T
    return out


def run_cores(in_maps, trace=False):
    from concourse.bass_utils import run_bass_kernel_spmd
    nc = _get_nc()
    return run_bass_kernel_spmd(nc, in_maps, list(range(NCORES)), trace=trace)


def kernel(x, Wq, Wk, Wv):
    in_maps = _host_inputs(x, Wq, Wk, Wv)
    res = run_cores(in_maps, trace=False)
    return _assemble(res.results)
